# revision 1
# baseline (speedup 1.0000x reference)
"""Trainium2 Bass kernel for GQA attention (QK-RMSNorm + RoPE + softmax + o_proj).

Sharding over 8 NeuronCores: core = (batch b in {0,1}, sequence quarter sc in
{0..3}).  Each core produces the final output rows for its 512 queries:
  - K/V projections for its own 512 rows (all 4 kv heads), RMSNorm+RoPE on K,
  - AllGather of K^T/V across the 4 cores of the same batch -> full-S K/V,
  - Q projection (all 16 heads) + RMSNorm + RoPE,
  - scores^T tiles = K^T_tile.T @ Q^T  (exp without max-subtraction: RMSNorm
    bounds |logit| <= sqrt(128) * max|w|^2),
  - row-sums via ones-matmul, AV accumulation, 1/sum scaling,
  - o_proj (contraction over all heads) -> out rows [512, 2048].

All matmuls run in float32r (full PE rate for moving dim >= 256, ~1.5e-4
rounding).  Elementwise math stays float32; values are rounded to f32r only on
the final write before each matmul operand.

kernel(**inputs) takes FULL unsharded inputs, returns the full output.
Host-side prep (weight transposes, slicing) is numpy; device time is the
graded kernel.
"""
import contextlib

import numpy as np

import concourse.bass as bass
import concourse.mybir as mybir
import concourse.tile as tile
from concourse import bacc
from concourse.bass_utils import run_bass_kernel_spmd

B, S, HID = 2, 2048, 2048
NH, NKV, D = 16, 4, 128
SC = 512           # per-core sequence chunk (queries)
KT_H = HID // 128  # 16 contraction tiles over hidden dim
EPS = 1e-6
INV_SQRT_D = 1.0 / float(np.sqrt(D))
GROUP_KT = 2       # score k-tiles per psum group (2 banks per group)

F32R = mybir.dt.float32r
F32 = mybir.dt.float32


def build_nc(mode="real", max_iters=64, upto="full"):
    """mode: 'real' (with AllGather) or 'timed' (AllGather emulated by local
    DMA copies, body wrapped in a runtime-count For_i hardware loop)."""
    nc = bacc.Bacc("TRN2", target_bir_lowering=False, debug=False, num_devices=8)

    d = {}
    d["xT"] = nc.dram_tensor("xT", [HID, SC], F32R, kind="ExternalInput")
    d["wqT"] = nc.dram_tensor("wqT", [HID, NH * D], F32R, kind="ExternalInput")
    d["wkT"] = nc.dram_tensor("wkT", [HID, NKV * D], F32R, kind="ExternalInput")
    d["wvT"] = nc.dram_tensor("wvT", [HID, NKV * D], F32R, kind="ExternalInput")
    d["woT"] = nc.dram_tensor("woT", [NH * D, HID], F32R, kind="ExternalInput")
    d["cosT"] = nc.dram_tensor("cosT", [D, SC], F32, kind="ExternalInput")
    d["sinpT"] = nc.dram_tensor("sinpT", [D, SC], F32, kind="ExternalInput")
    d["qw"] = nc.dram_tensor("qw", [D, 1], F32, kind="ExternalInput")
    d["kw"] = nc.dram_tensor("kw", [D, 1], F32, kind="ExternalInput")
    d["rmat"] = nc.dram_tensor("rmat", [D, D], F32R, kind="ExternalInput")
    d["onesc"] = nc.dram_tensor("onesc", [128, 1], F32R, kind="ExternalInput")
    d["onesr"] = nc.dram_tensor("onesr", [1, 128], F32R, kind="ExternalInput")
    d["epsc"] = nc.dram_tensor("epsc", [1, 1], F32, kind="ExternalInput")
    d["out"] = nc.dram_tensor("out", [SC, HID], F32, kind="ExternalOutput")
    if mode == "timed":
        d["nit"] = nc.dram_tensor("nit", [1, 1], mybir.dt.int32, kind="ExternalInput")

    d["kv_local"] = nc.dram_tensor("kv_local", [SC, 1024], F32R)
    d["kv_all"] = nc.dram_tensor("kv_all", [4 * SC, 1024], F32R)

    with tile.TileContext(nc) as tc, \
         nc.allow_low_precision(reason="float32r is 4-byte fp32"):
        with contextlib.ExitStack() as ctx:
            cpool = ctx.enter_context(tc.tile_pool(name="consts", bufs=1))
            qt_pool = ctx.enter_context(tc.tile_pool(name="qt", bufs=1))

            c = {}
            for nm, shape, dt_ in [("cosT", [D, SC], F32), ("sinpT", [D, SC], F32),
                                   ("qw", [D, 1], F32), ("kw", [D, 1], F32),
                                   ("rmat", [D, D], F32R),
                                   ("onesc", [128, 1], F32R),
                                   ("onesr", [1, 128], F32R),
                                   ("epsc", [1, 1], F32)]:
                c[nm] = cpool.tile(shape, dt_, name=f"c_{nm}")
                nc.sync.dma_start(out=c[nm][:], in_=d[nm][:])

            QT = qt_pool.tile([128, NH * SC], F32R)  # [d, h*SC + sq]

            if mode == "timed":
                nit_sb = cpool.tile([1, 1], mybir.dt.int32)
                nc.sync.dma_start(out=nit_sb[:], in_=d["nit"][:])
                with tc.tile_critical():
                    regs = []
                    for e in mybir.ALL_ENGINES:
                        eng = nc.engines[e]
                        tmp = eng.alloc_register(f"nit_{e.name}")
                        eng.reg_load(tmp, nit_sb[0:1, 0:1])
                        regs.append(tmp)
                    n_val = nc.snap(bass.RegisterHandles(regs), donate=True,
                                    min_val=0, max_val=max_iters)
                loop_cm = tc.For_i(0, n_val, 1)
            else:
                loop_cm = contextlib.nullcontext()

            with loop_cm:
                _emit_body(nc, tc, mode, d, c, QT, upto)
                if mode == "timed":
                    dummy = cpool.tile([1, 8], F32)
                    nc.gpsimd.memset(dummy[:], 0.0)

    nc.compile()
    return nc


def _emit_body(nc, tc, mode, d, c, QT, upto="full"):
    if upto == "dma":
        _emit_dma_only(nc, tc, mode, d, c)
        return
    # ---------------- projections ----------------
    with contextlib.ExitStack() as ctx:
        hs_pool = ctx.enter_context(tc.tile_pool(name="hsT", bufs=1))
        wsl_pool = ctx.enter_context(tc.tile_pool(name="wsl", bufs=1))
        rope_pool = ctx.enter_context(tc.tile_pool(name="rope", bufs=1))
        pp = ctx.enter_context(tc.tile_pool(name="pproj", bufs=1, space="PSUM"))

        def rope_block(psum_q, wcol, dst, dst_col):
            """RMSNorm(+w) and RoPE on psum_q [128 d, SC]; f32r write to
            dst[:, dst_col:dst_col+SC]."""
            sq = rope_pool.tile([128, SC], F32R, tag="sq", bufs=2, name="sq")
            nc.scalar.square(sq[:], psum_q[:])
            psA = pp.tile([1, SC], F32, tag="pssum", bufs=1, name="psA")
            nc.tensor.matmul(psA[:], c["onesc"][:], sq[:], start=True, stop=True)
            rrow = rope_pool.tile([1, SC], F32, tag="rrow", bufs=2, name="rrow")
            nc.scalar.activation(rrow[:], psA[:], mybir.ActivationFunctionType.Sqrt,
                                 bias=c["epsc"][:], scale=1.0 / D)
            rrec = rope_pool.tile([1, SC], F32R, tag="rrec", bufs=2, name="rrec")
            nc.vector.reciprocal(rrec[:], rrow[:])
            psR = pp.tile([128, SC], F32, tag="pr", bufs=1, name="psR")
            nc.tensor.matmul(psR[:], c["onesr"][:], rrec[:], start=True, stop=True)
            qw_t = rope_pool.tile([128, SC], F32R, tag="qwt", bufs=2, name="qwt")
            nc.vector.tensor_scalar_mul(qw_t[:], psum_q[:], wcol[:])
            prot = pp.tile([128, SC], F32, tag="prot", bufs=1, name="prot")
            nc.tensor.matmul(prot[:], c["rmat"][:], qw_t[:], start=True, stop=True)
            a_t = rope_pool.tile([128, SC], F32, tag="a", bufs=2, name="a_t")
            nc.vector.tensor_mul(a_t[:], qw_t[:], c["cosT"][:])
            b_t = rope_pool.tile([128, SC], F32, tag="b", bufs=2, name="b_t")
            nc.vector.tensor_mul(b_t[:], prot[:], c["sinpT"][:])
            ab_t = rope_pool.tile([128, SC], F32, tag="ab", bufs=2, name="ab_t")
            nc.vector.tensor_add(ab_t[:], a_t[:], b_t[:])
            nc.vector.tensor_mul(dst[:, dst_col:dst_col + SC], ab_t[:], psR[:])

        # hidden states (pre-transposed on host): 16 tiles [128, SC]
        hsT = hs_pool.tile([128, KT_H * SC], F32R)
        for kt in range(KT_H):
            nc.sync.dma_start(out=hsT[:, kt * SC:(kt + 1) * SC],
                              in_=d["xT"][kt * 128:(kt + 1) * 128, :])
        # resident K/V weights (4 MB each)
        wk_full = hs_pool.tile([128, KT_H * NKV * D], F32R)
        for kt in range(KT_H):
            nc.sync.dma_start(out=wk_full[:, kt * 512:(kt + 1) * 512],
                              in_=d["wkT"][kt * 128:(kt + 1) * 128, :])
        wv_full = hs_pool.tile([128, KT_H * NKV * D], F32R)
        for kt in range(KT_H):
            nc.sync.dma_start(out=wv_full[:, kt * 512:(kt + 1) * 512],
                              in_=d["wvT"][kt * 128:(kt + 1) * 128, :])

        # K proj + norm/rope -> kv_local[:, 0:512]
        for kvh in range(NKV):
            psk = pp.tile([128, SC], F32, tag="pq", bufs=4, name="psk")
            for kt in range(KT_H):
                nc.tensor.matmul(
                    psk[:],
                    wk_full[:, kt * 512 + kvh * D: kt * 512 + (kvh + 1) * D],
                    hsT[:, kt * SC:(kt + 1) * SC],
                    start=(kt == 0), stop=(kt == KT_H - 1))
            ktile = rope_pool.tile([128, SC], F32R, tag="kvtmp", bufs=3, name="ktile")
            rope_block(psk, c["kw"], ktile, 0)
            nc.sync.dma_start(out=d["kv_local"][kvh * 128:(kvh + 1) * 128, 0:512],
                              in_=ktile[:])

        # V proj -> kv_local[:, 512:1024]
        for st in range(4):
            psv = pp.tile([128, SC], F32, tag="pq", bufs=4, name="psv")
            for kt in range(KT_H):
                nc.tensor.matmul(
                    psv[:],
                    hsT[:, kt * SC + st * 128: kt * SC + (st + 1) * 128],
                    wv_full[:, kt * 512:(kt + 1) * 512],
                    start=(kt == 0), stop=(kt == KT_H - 1))
            vtile = rope_pool.tile([128, SC], F32R, tag="kvtmp", bufs=3, name="vtile")
            nc.vector.tensor_copy(vtile[:], psv[:])
            nc.sync.dma_start(out=d["kv_local"][st * 128:(st + 1) * 128, 512:1024],
                              in_=vtile[:])

        # AllGather K/V across the 4 cores of this batch
        if mode == "real":
            nc.gpsimd.collective_compute(
                "AllGather", mybir.AluOpType.bypass,
                ins=[d["kv_local"][:]], outs=[d["kv_all"][:]],
                replica_groups=[[0, 1, 2, 3], [4, 5, 6, 7]])
        else:
            for r in range(4):
                nc.sync.dma_start(out=d["kv_all"][r * SC:(r + 1) * SC, :],
                                  in_=d["kv_local"][:])

        # Q proj + norm/rope -> QT   (heads in pairs; WqT streamed in slices)
        for hp in range(NH // 2):
            wq_sl = wsl_pool.tile([128, KT_H * 2 * D], F32R, tag="wq", bufs=2,
                                  name="wq_sl")
            for kt in range(KT_H):
                nc.sync.dma_start(
                    out=wq_sl[:, kt * 256:(kt + 1) * 256],
                    in_=d["wqT"][kt * 128:(kt + 1) * 128,
                                 hp * 256:(hp + 1) * 256])
            for j in range(2):
                h = 2 * hp + j
                psq = pp.tile([128, SC], F32, tag="pq", bufs=4, name="psq")
                for kt in range(KT_H):
                    nc.tensor.matmul(
                        psq[:],
                        wq_sl[:, kt * 256 + j * D: kt * 256 + (j + 1) * D],
                        hsT[:, kt * SC:(kt + 1) * SC],
                        start=(kt == 0), stop=(kt == KT_H - 1))
                rope_block(psq, c["qw"], QT, h * SC)

    if upto == "proj":
        with tc.tile_pool(name="fin", bufs=1) as fin:
            ft = fin.tile([128, 512], F32)
            nc.vector.tensor_copy(ft[:], QT[:, 0:512])
            nc.sync.dma_start(out=d["out"][0:128, 0:512], in_=ft[:])
        return

    # ---------------- attention ----------------
    with contextlib.ExitStack() as ctx:
        kv_pool = ctx.enter_context(tc.tile_pool(name="kv", bufs=1))
        pt_pool = ctx.enter_context(tc.tile_pool(name="pt", bufs=1))
        avt_pool = ctx.enter_context(tc.tile_pool(name="avt", bufs=1))
        sm_pool = ctx.enter_context(tc.tile_pool(name="sm", bufs=1))
        pa = ctx.enter_context(tc.tile_pool(name="pattn", bufs=1, space="PSUM"))

        # K^T readback: [128 d, kvh*2048 + sk]
        KTs = kv_pool.tile([128, NKV * S], F32R, tag="KTs")
        for kvh in range(NKV):
            for r in range(4):
                nc.sync.dma_start(
                    out=KTs[:, kvh * S + r * 512: kvh * S + (r + 1) * 512],
                    in_=d["kv_all"][r * SC + kvh * 128: r * SC + (kvh + 1) * 128,
                                    0:512])
        # V readback: [128 sk_in_tile, kvh*2048 + t*128 + dv]
        Vs = kv_pool.tile([128, NKV * S], F32R, tag="Vs")
        for kvh in range(NKV):
            for r in range(4):
                nc.sync.dma_start(
                    out=Vs[:, kvh * S + r * 512: kvh * S + (r + 1) * 512]
                        .rearrange("p (t dv) -> p t dv", t=4),
                    in_=d["kv_all"][r * SC:(r + 1) * SC,
                                    512 + kvh * D: 512 + (kvh + 1) * D]
                        .rearrange("(t p) dv -> p t dv", p=128))

        AVT = avt_pool.tile([128, NH * SC], F32R)  # [d, h*SC + sq]
        # process q-heads in pairs sharing the kv head: consecutive matmuls
        # share the stationary operand (K^T tile / V tile), and the exp row
        # sums come from a DVE-accumulated sum of the exp tiles (16 ones-
        # matmuls total instead of 256).
        for grp in range(NH // 2):
            kvh = grp // 2
            h0 = 2 * grp
            pav = [pa.tile([128, SC], F32, tag=f"pavt{j}", bufs=1,
                           name=f"pav{j}") for j in range(2)]
            ptacc = sm_pool.tile([128, 2 * SC], F32, tag="ptacc", bufs=2,
                                 name="ptacc")
            for t in range(16):
                psc = pa.tile([128, 2 * SC], F32, tag="psc", bufs=2, name="psc")
                for j in range(2):
                    nc.tensor.matmul(
                        psc[:, j * SC:(j + 1) * SC],
                        KTs[:, kvh * S + t * 128: kvh * S + (t + 1) * 128],
                        QT[:, (h0 + j) * SC:(h0 + j + 1) * SC],
                        start=True, stop=True)
                pt_t = pt_pool.tile([128, 2 * SC], F32R, tag="pt", bufs=3,
                                    name="pt_t")
                nc.scalar.activation(pt_t[:], psc[:],
                                     mybir.ActivationFunctionType.Exp,
                                     bias=0.0, scale=INV_SQRT_D)
                for j in range(2):
                    nc.tensor.matmul(
                        pav[j][:],
                        Vs[:, kvh * S + t * 128: kvh * S + (t + 1) * 128],
                        pt_t[:, j * SC:(j + 1) * SC],
                        start=(t == 0), stop=(t == 15), skip_group_check=True)
                if t == 0:
                    nc.vector.tensor_copy(ptacc[:], pt_t[:])
                else:
                    nc.vector.tensor_add(ptacc[:], ptacc[:], pt_t[:])
            ptacc_r = sm_pool.tile([128, 2 * SC], F32R, tag="ptaccr", bufs=2,
                                   name="ptacc_r")
            nc.vector.tensor_copy(ptacc_r[:], ptacc[:])
            for j in range(2):
                h = h0 + j
                prow = pa.tile([1, SC], F32, tag="prow", bufs=1, name="prow")
                nc.tensor.matmul(prow[:], c["onesc"][:],
                                 ptacc_r[:, j * SC:(j + 1) * SC],
                                 start=True, stop=True)
                srec = sm_pool.tile([1, SC], F32R, tag="srec", bufs=2, name="srec")
                nc.vector.reciprocal(srec[:], prow[:])
                psR2 = pa.tile([128, SC], F32, tag="pr2", bufs=1, name="psR2")
                nc.tensor.matmul(psR2[:], c["onesr"][:], srec[:],
                                 start=True, stop=True)
                rb = sm_pool.tile([128, SC], F32, tag="rb", bufs=2, name="rb")
                nc.vector.tensor_copy(rb[:], psR2[:])
                nc.vector.tensor_mul(AVT[:, h * SC:(h + 1) * SC], pav[j][:], rb[:])

    if upto == "attn":
        with tc.tile_pool(name="fin", bufs=1) as fin:
            ft = fin.tile([128, 512], F32)
            nc.vector.memset(ft[:], 0.0)
            nc.sync.dma_start(out=d["out"][0:128, 0:512], in_=ft[:])
        return

    # ---------------- o_proj ----------------
    with contextlib.ExitStack() as ctx:
        wo_pool = ctx.enter_context(tc.tile_pool(name="wo", bufs=1))
        oacc_pool = ctx.enter_context(tc.tile_pool(name="oacc", bufs=1))
        po_pool = ctx.enter_context(tc.tile_pool(name="po", bufs=1, space="PSUM"))

        out_acc = oacc_pool.tile([128, 4 * HID], F32)  # [s%128, st*HID + Hcol]
        for rnd in range(4):
            wo_ts = []
            for j in range(4):
                h = 4 * rnd + j
                wo_t = wo_pool.tile([128, HID], F32R, tag="wo", bufs=5,
                                    name=f"wo_t{h}")
                nc.sync.dma_start(out=wo_t[:], in_=d["woT"][h * 128:(h + 1) * 128, :])
                wo_ts.append(wo_t)
            for st in range(4):
                pos = [po_pool.tile([128, 512], F32, tag=f"po{hc}", bufs=2,
                                    name=f"po{hc}") for hc in range(4)]
                for j in range(4):
                    h = 4 * rnd + j
                    for hc in range(4):
                        nc.tensor.matmul(
                            pos[hc][:],
                            AVT[:, h * SC + st * 128: h * SC + (st + 1) * 128],
                            wo_ts[j][:, hc * 512:(hc + 1) * 512],
                            start=(j == 0), stop=(j == 3))
                for hc in range(4):
                    dst = out_acc[:, st * HID + hc * 512: st * HID + (hc + 1) * 512]
                    if rnd == 0:
                        nc.vector.tensor_copy(dst, pos[hc][:])
                    else:
                        nc.vector.tensor_add(dst, dst, pos[hc][:])
        for st in range(4):
            nc.sync.dma_start(out=d["out"][st * 128:(st + 1) * 128, :],
                              in_=out_acc[:, st * HID:(st + 1) * HID])


def _emit_dma_only(nc, tc, mode, d, c):
    with contextlib.ExitStack() as ctx:
        hs_pool = ctx.enter_context(tc.tile_pool(name="hsT", bufs=1))
        wsl_pool = ctx.enter_context(tc.tile_pool(name="wsl", bufs=1))
        kv_pool = ctx.enter_context(tc.tile_pool(name="kv", bufs=1))
        wo_pool = ctx.enter_context(tc.tile_pool(name="wo", bufs=1))
        oacc_pool = ctx.enter_context(tc.tile_pool(name="oacc", bufs=1))

        hsT = hs_pool.tile([128, KT_H * SC], F32R)
        for kt in range(KT_H):
            nc.sync.dma_start(out=hsT[:, kt * SC:(kt + 1) * SC],
                              in_=d["xT"][kt * 128:(kt + 1) * 128, :])
        wk_full = hs_pool.tile([128, KT_H * NKV * D], F32R)
        for kt in range(KT_H):
            nc.sync.dma_start(out=wk_full[:, kt * 512:(kt + 1) * 512],
                              in_=d["wkT"][kt * 128:(kt + 1) * 128, :])
        wv_full = hs_pool.tile([128, KT_H * NKV * D], F32R)
        for kt in range(KT_H):
            nc.sync.dma_start(out=wv_full[:, kt * 512:(kt + 1) * 512],
                              in_=d["wvT"][kt * 128:(kt + 1) * 128, :])
        for hp in range(NH // 2):
            wq_sl = wsl_pool.tile([128, KT_H * 2 * D], F32R, tag="wq", bufs=2,
                                  name="wq_sl")
            for kt in range(KT_H):
                nc.sync.dma_start(
                    out=wq_sl[:, kt * 256:(kt + 1) * 256],
                    in_=d["wqT"][kt * 128:(kt + 1) * 128,
                                 hp * 256:(hp + 1) * 256])
        # kv_local writes (from hsT slices, any data) + AG emu + readbacks
        for i in range(8):
            nc.sync.dma_start(out=d["kv_local"][i * 64:(i + 1) * 64, :],
                              in_=hsT[0:64, 0:1024])
        for r in range(4):
            nc.sync.dma_start(out=d["kv_all"][r * SC:(r + 1) * SC, :],
                              in_=d["kv_local"][:])
        KTs = kv_pool.tile([128, NKV * S], F32R, tag="KTs")
        for kvh in range(NKV):
            for r in range(4):
                nc.sync.dma_start(
                    out=KTs[:, kvh * S + r * 512: kvh * S + (r + 1) * 512],
                    in_=d["kv_all"][r * SC + kvh * 128: r * SC + (kvh + 1) * 128,
                                    0:512])
        Vs = kv_pool.tile([128, NKV * S], F32R, tag="Vs")
        for kvh in range(NKV):
            for r in range(4):
                nc.sync.dma_start(
                    out=Vs[:, kvh * S + r * 512: kvh * S + (r + 1) * 512]
                        .rearrange("p (t dv) -> p t dv", t=4),
                    in_=d["kv_all"][r * SC:(r + 1) * SC,
                                    512 + kvh * D: 512 + (kvh + 1) * D]
                        .rearrange("(t p) dv -> p t dv", p=128))
        for h in range(NH):
            wo_t = wo_pool.tile([128, HID], F32R, tag="wo", bufs=5,
                                name=f"wo_t{h}")
            nc.sync.dma_start(out=wo_t[:], in_=d["woT"][h * 128:(h + 1) * 128, :])
        out_acc = oacc_pool.tile([128, 4 * HID], F32)
        nc.vector.memset(out_acc[:, 0:8192], 0.0)
        for st in range(4):
            nc.sync.dma_start(out=d["out"][st * 128:(st + 1) * 128, :],
                              in_=out_acc[:, st * HID:(st + 1) * HID])


def host_prep(hidden_states, cos, sin, Wq, Wk, Wv, Wo, q_norm_w, k_norm_w):
    """Build the 8 per-core input maps (host-side layout prep)."""
    hs = np.asarray(hidden_states, dtype=np.float32)
    cos = np.asarray(cos, dtype=np.float32)
    sin = np.asarray(sin, dtype=np.float32)
    sinp = np.concatenate([-sin[..., :64], sin[..., 64:]], axis=-1)
    wqT = np.ascontiguousarray(np.asarray(Wq, np.float32).T)
    wkT = np.ascontiguousarray(np.asarray(Wk, np.float32).T)
    wvT = np.ascontiguousarray(np.asarray(Wv, np.float32).T)
    woT = np.ascontiguousarray(np.asarray(Wo, np.float32).T)
    rmat = np.zeros((D, D), np.float32)
    rmat[(np.arange(D) + 64) % D, np.arange(D)] = 1.0
    onesc = np.ones((128, 1), np.float32)
    onesr = np.ones((1, 128), np.float32)
    qwc = np.asarray(q_norm_w, np.float32).reshape(D, 1)
    kwc = np.asarray(k_norm_w, np.float32).reshape(D, 1)

    in_maps = []
    for core in range(8):
        b, sc = divmod(core, 4)
        sl = slice(sc * SC, (sc + 1) * SC)
        in_maps.append({
            "xT": np.ascontiguousarray(hs[b, sl].T),
            "wqT": wqT, "wkT": wkT, "wvT": wvT, "woT": woT,
            "cosT": np.ascontiguousarray(cos[b, sl].T),
            "sinpT": np.ascontiguousarray(sinp[b, sl].T),
            "qw": qwc, "kw": kwc,
            "rmat": rmat, "onesc": onesc, "onesr": onesr,
            "epsc": np.full((1, 1), EPS, np.float32),
        })
    return in_maps


_nc_cache = {}


def get_nc(mode="real"):
    if mode not in _nc_cache:
        _nc_cache[mode] = build_nc(mode)
    return _nc_cache[mode]


def kernel(**inputs) -> np.ndarray:
    nc = get_nc("real")
    in_maps = host_prep(**inputs)
    res = run_bass_kernel_spmd(nc, in_maps, list(range(8)))
    out = np.empty((B, S, HID), np.float32)
    for core in range(8):
        b, sc = divmod(core, 4)
        out[b, sc * SC:(sc + 1) * SC, :] = res.results[core]["out"]
    return out


if __name__ == "__main__":
    import reference
    inputs = {k: np.asarray(v) for k, v in reference.setup_inputs().items()}
    expected = np.asarray(reference.reference(**inputs))
    actual = kernel(**inputs)
    err = np.abs(actual - expected)
    rel = err.max() / np.abs(expected).max()
    print(f"max abs err {err.max():.3e}  rel (vs absmax) {rel:.3e}")



# revision 5
# speedup vs baseline: 6.7405x; 6.7405x over previous
"""Trainium2 Bass kernel for GQA attention (QK-RMSNorm + RoPE + softmax + o_proj).

Sharding over 8 NeuronCores: core = (batch b in {0,1}, sequence quarter sc in
{0..3}).  Each core produces the final output rows for its 512 queries:
  - K/V projections for its own 512 rows (all 4 kv heads), RMSNorm+RoPE on K,
  - AllGather of K^T/V across the 4 cores of the same batch -> full-S K/V,
  - Q projection (all 16 heads) + RMSNorm + RoPE,
  - scores^T tiles = K^T_tile.T @ Q^T  (exp without max-subtraction: RMSNorm
    bounds |logit| <= sqrt(128) * max|w|^2),
  - row-sums via DVE accumulation + ones-matmul, AV accumulation, 1/sum scale,
  - o_proj (PSUM-accumulated over all 16 heads, DMA'd straight to DRAM).

All matmul operands are bfloat16 (PE full rate, half the DMA/SBUF of f32);
accumulation stays f32 in PSUM.  Elementwise intermediates are bf16 where both
operands allow the DVE 2x mode.

kernel(**inputs) takes FULL unsharded inputs, returns the full output.
Host-side prep (weight transposes, slicing, bf16 cast) is numpy; device time
is the graded kernel.
"""
import contextlib

import ml_dtypes
import numpy as np

import concourse.bass as bass
import concourse.mybir as mybir
import concourse.tile as tile
from concourse import bacc
from concourse.bass_utils import run_bass_kernel_spmd

B, S, HID = 2, 2048, 2048
NH, NKV, D = 16, 4, 128
SC = 512           # per-core sequence chunk (queries)
KT_H = HID // 128  # 16 contraction tiles over hidden dim
EPS = 1e-6
INV_SQRT_D = 1.0 / float(np.sqrt(D))

BF16 = mybir.dt.bfloat16
F32 = mybir.dt.float32
NP_BF16 = ml_dtypes.bfloat16


def build_nc(mode="real", max_iters=64, upto="full"):
    """mode: 'real' (with AllGather), 'timed' (AllGather emulated by local
    DMA copies, body wrapped in a runtime-count For_i hardware loop), or
    'sim' (AllGather emulated, no loop — for TimelineSim)."""
    nc = bacc.Bacc("TRN2", target_bir_lowering=False, debug=False, num_devices=8)

    d = {}
    d["xT"] = nc.dram_tensor("xT", [HID, SC], BF16, kind="ExternalInput")
    d["wqT"] = nc.dram_tensor("wqT", [HID, NH * D], BF16, kind="ExternalInput")
    d["wkT"] = nc.dram_tensor("wkT", [HID, NKV * D], BF16, kind="ExternalInput")
    d["wvT"] = nc.dram_tensor("wvT", [HID, NKV * D], BF16, kind="ExternalInput")
    d["woT"] = nc.dram_tensor("woT", [NH * D, HID], BF16, kind="ExternalInput")
    d["cosT"] = nc.dram_tensor("cosT", [D, SC], BF16, kind="ExternalInput")
    d["sinpT"] = nc.dram_tensor("sinpT", [D, SC], BF16, kind="ExternalInput")
    d["qw"] = nc.dram_tensor("qw", [D, 1], F32, kind="ExternalInput")
    d["kw"] = nc.dram_tensor("kw", [D, 1], F32, kind="ExternalInput")
    d["rmat"] = nc.dram_tensor("rmat", [D, D], BF16, kind="ExternalInput")
    d["onesc"] = nc.dram_tensor("onesc", [128, 1], BF16, kind="ExternalInput")
    d["onesr"] = nc.dram_tensor("onesr", [1, 128], BF16, kind="ExternalInput")
    d["epsc"] = nc.dram_tensor("epsc", [1, 1], F32, kind="ExternalInput")
    d["out"] = nc.dram_tensor("out", [SC, HID], F32, kind="ExternalOutput")
    if mode == "timed":
        d["nit"] = nc.dram_tensor("nit", [1, 1], mybir.dt.int32, kind="ExternalInput")

    d["kv_local"] = nc.dram_tensor("kv_local", [SC, 1024], BF16)
    d["kv_all"] = nc.dram_tensor("kv_all", [4 * SC, 1024], BF16)

    with tile.TileContext(nc) as tc, \
         nc.allow_low_precision(reason="bf16 operands, f32 accumulation"):
        with contextlib.ExitStack() as ctx:
            cpool = ctx.enter_context(tc.tile_pool(name="consts", bufs=1))
            qt_pool = ctx.enter_context(tc.tile_pool(name="qt", bufs=1))

            c = {}
            for nm, shape, dt_ in [("cosT", [D, SC], BF16), ("sinpT", [D, SC], BF16),
                                   ("qw", [D, 1], F32), ("kw", [D, 1], F32),
                                   ("rmat", [D, D], BF16),
                                   ("onesc", [128, 1], BF16),
                                   ("onesr", [1, 128], BF16),
                                   ("epsc", [1, 1], F32)]:
                c[nm] = cpool.tile(shape, dt_, name=f"c_{nm}")
                nc.sync.dma_start(out=c[nm][:], in_=d[nm][:])

            QT = qt_pool.tile([128, NH * SC], BF16)  # [d, h*SC + sq]

            if mode == "timed":
                nit_sb = cpool.tile([1, 1], mybir.dt.int32)
                nc.sync.dma_start(out=nit_sb[:], in_=d["nit"][:])
                with tc.tile_critical():
                    regs = []
                    for e in mybir.ALL_ENGINES:
                        eng = nc.engines[e]
                        tmp = eng.alloc_register(f"nit_{e.name}")
                        eng.reg_load(tmp, nit_sb[0:1, 0:1])
                        regs.append(tmp)
                    n_val = nc.snap(bass.RegisterHandles(regs), donate=True,
                                    min_val=0, max_val=max_iters)
                loop_cm = tc.For_i(0, n_val, 1)
            else:
                loop_cm = contextlib.nullcontext()

            with loop_cm:
                _emit_body(nc, tc, mode, d, c, QT, upto)
                if mode == "timed":
                    dummy = cpool.tile([1, 8], F32)
                    nc.gpsimd.memset(dummy[:], 0.0)

    nc.compile()
    return nc


def _emit_body(nc, tc, mode, d, c, QT, upto="full"):
    if upto == "dma":
        _emit_dma_only(nc, tc, mode, d, c)
        return
    # ---------------- projections ----------------
    with contextlib.ExitStack() as ctx:
        hs_pool = ctx.enter_context(tc.tile_pool(name="hsT", bufs=1))
        rope_pool = ctx.enter_context(tc.tile_pool(name="rope", bufs=1))
        pp = ctx.enter_context(tc.tile_pool(name="pproj", bufs=1, space="PSUM"))

        def rope_block(psum_q, wcol, dst, dst_col):
            """RMSNorm(+w) and RoPE on psum_q [128 d, SC]; bf16 write to
            dst[:, dst_col:dst_col+SC]."""
            sq = rope_pool.tile([128, SC], BF16, tag="sq", bufs=2, name="sq")
            nc.scalar.square(sq[:], psum_q[:])
            psA = pp.tile([1, SC], F32, tag="pssum", bufs=1, name="psA")
            nc.tensor.matmul(psA[:], c["onesc"][:], sq[:], start=True, stop=True)
            rrow = rope_pool.tile([1, SC], F32, tag="rrow", bufs=2, name="rrow")
            nc.scalar.activation(rrow[:], psA[:], mybir.ActivationFunctionType.Sqrt,
                                 bias=c["epsc"][:], scale=1.0 / D)
            rrec = rope_pool.tile([1, SC], BF16, tag="rrec", bufs=2, name="rrec")
            nc.vector.reciprocal(rrec[:], rrow[:])
            psR = pp.tile([128, SC], F32, tag="pr", bufs=1, name="psR")
            nc.tensor.matmul(psR[:], c["onesr"][:], rrec[:], start=True, stop=True)
            qw_t = rope_pool.tile([128, SC], BF16, tag="qwt", bufs=2, name="qwt")
            nc.vector.tensor_scalar_mul(qw_t[:], psum_q[:], wcol[:])
            prot = pp.tile([128, SC], F32, tag="prot", bufs=1, name="prot")
            nc.tensor.matmul(prot[:], c["rmat"][:], qw_t[:], start=True, stop=True)
            a_t = rope_pool.tile([128, SC], BF16, tag="a", bufs=2, name="a_t")
            nc.vector.tensor_mul(a_t[:], qw_t[:], c["cosT"][:])
            b_t = rope_pool.tile([128, SC], BF16, tag="b", bufs=2, name="b_t")
            nc.vector.tensor_mul(b_t[:], prot[:], c["sinpT"][:])
            ab_t = rope_pool.tile([128, SC], BF16, tag="ab", bufs=2, name="ab_t")
            nc.vector.tensor_add(ab_t[:], a_t[:], b_t[:])
            nc.vector.tensor_mul(dst[:, dst_col:dst_col + SC], ab_t[:], psR[:])

        # hidden states (pre-transposed on host): 16 tiles [128, SC]
        hsT = hs_pool.tile([128, KT_H * SC], BF16)
        for kt in range(KT_H):
            nc.sync.dma_start(out=hsT[:, kt * SC:(kt + 1) * SC],
                              in_=d["xT"][kt * 128:(kt + 1) * 128, :])
        # resident K/V/Q weights (bf16: 2 + 2 + 8 MB)
        wk_full = hs_pool.tile([128, KT_H * NKV * D], BF16)
        for kt in range(KT_H):
            nc.sync.dma_start(out=wk_full[:, kt * 512:(kt + 1) * 512],
                              in_=d["wkT"][kt * 128:(kt + 1) * 128, :])
        wv_full = hs_pool.tile([128, KT_H * NKV * D], BF16)
        for kt in range(KT_H):
            nc.sync.dma_start(out=wv_full[:, kt * 512:(kt + 1) * 512],
                              in_=d["wvT"][kt * 128:(kt + 1) * 128, :])
        wq_full = hs_pool.tile([128, KT_H * NH * D], BF16)
        for kt in range(KT_H):
            nc.sync.dma_start(out=wq_full[:, kt * 2048:(kt + 1) * 2048],
                              in_=d["wqT"][kt * 128:(kt + 1) * 128, :])

        # K proj + norm/rope -> kv_local[:, 0:512]
        for kvh in range(NKV):
            psk = pp.tile([128, SC], F32, tag="pq", bufs=4, name="psk")
            for kt in range(KT_H):
                nc.tensor.matmul(
                    psk[:],
                    wk_full[:, kt * 512 + kvh * D: kt * 512 + (kvh + 1) * D],
                    hsT[:, kt * SC:(kt + 1) * SC],
                    start=(kt == 0), stop=(kt == KT_H - 1))
            ktile = rope_pool.tile([128, SC], BF16, tag="kvtmp", bufs=3, name="ktile")
            rope_block(psk, c["kw"], ktile, 0)
            nc.sync.dma_start(out=d["kv_local"][kvh * 128:(kvh + 1) * 128, 0:512],
                              in_=ktile[:])

        # V proj -> kv_local[:, 512:1024]  ([seq, dv] layout, all 4 kv heads)
        for st in range(4):
            psv = pp.tile([128, SC], F32, tag="pq", bufs=4, name="psv")
            for kt in range(KT_H):
                nc.tensor.matmul(
                    psv[:],
                    hsT[:, kt * SC + st * 128: kt * SC + (st + 1) * 128],
                    wv_full[:, kt * 512:(kt + 1) * 512],
                    start=(kt == 0), stop=(kt == KT_H - 1))
            vtile = rope_pool.tile([128, SC], BF16, tag="kvtmp", bufs=3, name="vtile")
            nc.vector.tensor_copy(vtile[:], psv[:])
            nc.sync.dma_start(out=d["kv_local"][st * 128:(st + 1) * 128, 512:1024],
                              in_=vtile[:])

        # AllGather K/V across the 4 cores of this batch
        if mode == "real":
            nc.gpsimd.collective_compute(
                "AllGather", mybir.AluOpType.bypass,
                ins=[d["kv_local"][:]], outs=[d["kv_all"][:]],
                replica_groups=[[0, 1, 2, 3], [4, 5, 6, 7]])
        else:
            for r in range(4):
                nc.sync.dma_start(out=d["kv_all"][r * SC:(r + 1) * SC, :],
                                  in_=d["kv_local"][:])

        # Q proj + norm/rope -> QT
        for h in range(NH):
            psq = pp.tile([128, SC], F32, tag="pq", bufs=4, name="psq")
            for kt in range(KT_H):
                nc.tensor.matmul(
                    psq[:],
                    wq_full[:, kt * 2048 + h * D: kt * 2048 + (h + 1) * D],
                    hsT[:, kt * SC:(kt + 1) * SC],
                    start=(kt == 0), stop=(kt == KT_H - 1))
            rope_block(psq, c["qw"], QT, h * SC)

    if upto == "proj":
        with tc.tile_pool(name="fin", bufs=1) as fin:
            ft = fin.tile([128, 512], F32)
            nc.vector.tensor_copy(ft[:], QT[:, 0:512])
            nc.sync.dma_start(out=d["out"][0:128, 0:512], in_=ft[:])
        return

    # ---------------- attention ----------------
    with contextlib.ExitStack() as ctx:
        kv_pool = ctx.enter_context(tc.tile_pool(name="kv", bufs=1))
        wo_pool = ctx.enter_context(tc.tile_pool(name="wo", bufs=1))
        pt_pool = ctx.enter_context(tc.tile_pool(name="pt", bufs=1))
        avt_pool = ctx.enter_context(tc.tile_pool(name="avt", bufs=1))
        sm_pool = ctx.enter_context(tc.tile_pool(name="sm", bufs=1))
        pa = ctx.enter_context(tc.tile_pool(name="pattn", bufs=1, space="PSUM"))

        # K^T readback: [128 d, kvh*2048 + sk]
        KTs = kv_pool.tile([128, NKV * S], BF16, tag="KTs")
        for kvh in range(NKV):
            for r in range(4):
                nc.sync.dma_start(
                    out=KTs[:, kvh * S + r * 512: kvh * S + (r + 1) * 512],
                    in_=d["kv_all"][r * SC + kvh * 128: r * SC + (kvh + 1) * 128,
                                    0:512])
        # V readback: [128 sk_in_tile, gt*512 + kvh*128 + dv]  (contiguous rows)
        Vs = kv_pool.tile([128, NKV * S], BF16, tag="Vs")
        for r in range(4):
            for tt in range(4):
                gt = r * 4 + tt
                nc.sync.dma_start(
                    out=Vs[:, gt * 512:(gt + 1) * 512],
                    in_=d["kv_all"][r * SC + tt * 128: r * SC + (tt + 1) * 128,
                                    512:1024])
        # o_proj weights: prefetch during attention.  block j = woT rows of
        # head j: [128 dv, HID]
        wo_full = wo_pool.tile([128, NH * HID], BF16)
        for j in range(NH):
            nc.sync.dma_start(out=wo_full[:, j * HID:(j + 1) * HID],
                              in_=d["woT"][j * 128:(j + 1) * 128, :])

        AVT = avt_pool.tile([128, NH * SC], BF16)  # [dv, h*SC + sq]
        # heads in pairs sharing the kv head: score/AV matmuls share the
        # stationary operand; exp + row-sum accumulation run batched over the
        # pair ([128, 1024] tiles).
        for grp in range(NH // 2):
            kvh = grp // 2
            h0 = 2 * grp
            pav = [pa.tile([128, SC], F32, tag=f"pavt{j}", bufs=1,
                           name=f"pav{j}") for j in range(2)]
            ptacc = sm_pool.tile([128, 2 * SC], BF16, tag="ptacc", bufs=2,
                                 name="ptacc")
            for t in range(16):
                psc = pa.tile([128, 2 * SC], F32, tag="psc", bufs=2, name="psc")
                for j in range(2):
                    nc.tensor.matmul(
                        psc[:, j * SC:(j + 1) * SC],
                        KTs[:, kvh * S + t * 128: kvh * S + (t + 1) * 128],
                        QT[:, (h0 + j) * SC:(h0 + j + 1) * SC],
                        start=True, stop=True)
                pt_t = pt_pool.tile([128, 2 * SC], BF16, tag="pt", bufs=3,
                                    name="pt_t")
                nc.scalar.activation(pt_t[:], psc[:],
                                     mybir.ActivationFunctionType.Exp,
                                     bias=0.0, scale=INV_SQRT_D)
                for j in range(2):
                    nc.tensor.matmul(
                        pav[j][:],
                        Vs[:, t * 512 + kvh * D: t * 512 + (kvh + 1) * D],
                        pt_t[:, j * SC:(j + 1) * SC],
                        start=(t == 0), stop=(t == 15), skip_group_check=True)
                if t == 0:
                    nc.vector.tensor_copy(ptacc[:], pt_t[:])
                else:
                    nc.vector.tensor_add(ptacc[:], ptacc[:], pt_t[:])
            for j in range(2):
                h = h0 + j
                prow = pa.tile([1, SC], F32, tag="prow", bufs=1, name="prow")
                nc.tensor.matmul(prow[:], c["onesc"][:],
                                 ptacc[:, j * SC:(j + 1) * SC],
                                 start=True, stop=True)
                srec = sm_pool.tile([1, SC], BF16, tag="srec", bufs=2,
                                    name="srec")
                nc.vector.reciprocal(srec[:], prow[:])
                psR2 = pa.tile([128, SC], F32, tag="pr2", bufs=1, name="psR2")
                nc.tensor.matmul(psR2[:], c["onesr"][:], srec[:],
                                 start=True, stop=True)
                rb = sm_pool.tile([128, SC], BF16, tag="rb", bufs=2, name="rb")
                nc.scalar.copy(rb[:], psR2[:])
                nc.vector.tensor_mul(AVT[:, h * SC:(h + 1) * SC], pav[j][:],
                                     rb[:])

    if upto == "attn":
        with tc.tile_pool(name="fin", bufs=1) as fin:
            ft = fin.tile([128, 512], F32)
            nc.vector.memset(ft[:], 0.0)
            nc.sync.dma_start(out=d["out"][0:128, 0:512], in_=ft[:])
        return

    # ---------------- o_proj ----------------
    # contraction over all 16 heads accumulated in PSUM; result DMA'd from
    # PSUM straight to DRAM.
    with contextlib.ExitStack() as ctx:
        po_pool = ctx.enter_context(tc.tile_pool(name="po", bufs=1, space="PSUM"))
        ost_pool = ctx.enter_context(tc.tile_pool(name="ost", bufs=1))
        for st in range(4):
            pos = [po_pool.tile([128, 512], F32, tag=f"po{hc}", bufs=2,
                                name=f"po{hc}") for hc in range(4)]
            for j in range(NH):
                for hc in range(4):
                    nc.tensor.matmul(
                        pos[hc][:],
                        AVT[:, j * SC + st * 128: j * SC + (st + 1) * 128],
                        wo_full[:, j * HID + hc * 512: j * HID + (hc + 1) * 512],
                        start=(j == 0), stop=(j == NH - 1),
                        skip_group_check=True)
            for hc in range(4):
                oct_ = ost_pool.tile([128, 512], F32, tag="oct", bufs=4,
                                     name="oct")
                nc.vector.tensor_copy(oct_[:], pos[hc][:])
                nc.sync.dma_start(
                    out=d["out"][st * 128:(st + 1) * 128,
                                 hc * 512:(hc + 1) * 512],
                    in_=oct_[:])


def _emit_dma_only(nc, tc, mode, d, c):
    with contextlib.ExitStack() as ctx:
        hs_pool = ctx.enter_context(tc.tile_pool(name="hsT", bufs=1))
        kv_pool = ctx.enter_context(tc.tile_pool(name="kv", bufs=1))
        wo_pool = ctx.enter_context(tc.tile_pool(name="wo", bufs=1))
        oacc_pool = ctx.enter_context(tc.tile_pool(name="oacc", bufs=1))

        hsT = hs_pool.tile([128, KT_H * SC], BF16)
        for kt in range(KT_H):
            nc.sync.dma_start(out=hsT[:, kt * SC:(kt + 1) * SC],
                              in_=d["xT"][kt * 128:(kt + 1) * 128, :])
        wk_full = hs_pool.tile([128, KT_H * NKV * D], BF16)
        for kt in range(KT_H):
            nc.sync.dma_start(out=wk_full[:, kt * 512:(kt + 1) * 512],
                              in_=d["wkT"][kt * 128:(kt + 1) * 128, :])
        wv_full = hs_pool.tile([128, KT_H * NKV * D], BF16)
        for kt in range(KT_H):
            nc.sync.dma_start(out=wv_full[:, kt * 512:(kt + 1) * 512],
                              in_=d["wvT"][kt * 128:(kt + 1) * 128, :])
        wq_full = hs_pool.tile([128, KT_H * NH * D], BF16)
        for kt in range(KT_H):
            nc.sync.dma_start(out=wq_full[:, kt * 2048:(kt + 1) * 2048],
                              in_=d["wqT"][kt * 128:(kt + 1) * 128, :])
        for i in range(8):
            nc.sync.dma_start(out=d["kv_local"][i * 64:(i + 1) * 64, :],
                              in_=hsT[0:64, 0:1024])
        for r in range(4):
            nc.sync.dma_start(out=d["kv_all"][r * SC:(r + 1) * SC, :],
                              in_=d["kv_local"][:])
        KTs = kv_pool.tile([128, NKV * S], BF16, tag="KTs")
        for kvh in range(NKV):
            for r in range(4):
                nc.sync.dma_start(
                    out=KTs[:, kvh * S + r * 512: kvh * S + (r + 1) * 512],
                    in_=d["kv_all"][r * SC + kvh * 128: r * SC + (kvh + 1) * 128,
                                    0:512])
        Vs = kv_pool.tile([128, NKV * S], BF16, tag="Vs")
        for r in range(4):
            for tt in range(4):
                gt = r * 4 + tt
                nc.sync.dma_start(
                    out=Vs[:, gt * 512:(gt + 1) * 512],
                    in_=d["kv_all"][r * SC + tt * 128: r * SC + (tt + 1) * 128,
                                    512:1024])
        wo_full = wo_pool.tile([128, NH * HID], BF16)
        for j in range(NH):
            nc.sync.dma_start(out=wo_full[:, j * HID:(j + 1) * HID],
                              in_=d["woT"][j * 128:(j + 1) * 128, :])
        out_acc = oacc_pool.tile([128, 4 * HID], F32)
        nc.vector.memset(out_acc[:, 0:8192], 0.0)
        for st in range(4):
            nc.sync.dma_start(out=d["out"][st * 128:(st + 1) * 128, :],
                              in_=out_acc[:, st * HID:(st + 1) * HID])


def host_prep(hidden_states, cos, sin, Wq, Wk, Wv, Wo, q_norm_w, k_norm_w):
    """Build the 8 per-core input maps (host-side layout prep + bf16 cast)."""
    hs = np.asarray(hidden_states, dtype=np.float32)
    cos = np.asarray(cos, dtype=np.float32)
    sin = np.asarray(sin, dtype=np.float32)
    sinp = np.concatenate([-sin[..., :64], sin[..., 64:]], axis=-1)
    wqT = np.ascontiguousarray(np.asarray(Wq, np.float32).T.astype(NP_BF16))
    wkT = np.ascontiguousarray(np.asarray(Wk, np.float32).T.astype(NP_BF16))
    wvT = np.ascontiguousarray(np.asarray(Wv, np.float32).T.astype(NP_BF16))
    woT = np.ascontiguousarray(np.asarray(Wo, np.float32).T.astype(NP_BF16))
    rmat = np.zeros((D, D), NP_BF16)
    rmat[(np.arange(D) + 64) % D, np.arange(D)] = 1.0
    onesc = np.ones((128, 1), NP_BF16)
    onesr = np.ones((1, 128), NP_BF16)
    qwc = np.asarray(q_norm_w, np.float32).reshape(D, 1)
    kwc = np.asarray(k_norm_w, np.float32).reshape(D, 1)

    in_maps = []
    for core in range(8):
        b, sc = divmod(core, 4)
        sl = slice(sc * SC, (sc + 1) * SC)
        in_maps.append({
            "xT": np.ascontiguousarray(hs[b, sl].T.astype(NP_BF16)),
            "wqT": wqT, "wkT": wkT, "wvT": wvT, "woT": woT,
            "cosT": np.ascontiguousarray(cos[b, sl].T.astype(NP_BF16)),
            "sinpT": np.ascontiguousarray(sinp[b, sl].T.astype(NP_BF16)),
            "qw": qwc, "kw": kwc,
            "rmat": rmat, "onesc": onesc, "onesr": onesr,
            "epsc": np.full((1, 1), EPS, np.float32),
        })
    return in_maps


_nc_cache = {}


def get_nc(mode="real"):
    if mode not in _nc_cache:
        _nc_cache[mode] = build_nc(mode)
    return _nc_cache[mode]


def kernel(**inputs) -> np.ndarray:
    nc = get_nc("real")
    in_maps = host_prep(**inputs)
    res = run_bass_kernel_spmd(nc, in_maps, list(range(8)))
    out = np.empty((B, S, HID), np.float32)
    for core in range(8):
        b, sc = divmod(core, 4)
        out[b, sc * SC:(sc + 1) * SC, :] = res.results[core]["out"]
    return out


if __name__ == "__main__":
    import reference
    inputs = {k: np.asarray(v) for k, v in reference.setup_inputs().items()}
    expected = np.asarray(reference.reference(**inputs))
    actual = kernel(**inputs)
    err = np.abs(actual - expected)
    rel = err.max() / np.abs(expected).max()
    print(f"max abs err {err.max():.3e}  rel (vs absmax) {rel:.3e}")
